# revision 15
# baseline (speedup 1.0000x reference)
"""APKDA loss (pool+normalize -> SmoothAP) as two distributed Bass launches on
8 TRN2 NeuronCores.

Math restructuring vs the reference:
  - Only the diagonal class-blocks of sim_all_rk are ever used, so per query q
    we need rank sums only over its 16 same-class columns j:
        r_all[q,j] = 1 + sum_k relu(S[q,k] - S[q,j])   (k over all 512 columns)
        r_pos[q,j] = 1 + sum_k relu(Sg[q,k] - Sg[q,j]) (k over the 16-group)
    with Sg the own-class block of S.  The eye-masks in the reference only
    kill k==j terms whose relu is 0 anyway.
  - L2-normalizing the hw-sum equals normalizing the hw-mean (scale cancels).
  - The key order of S is irrelevant (sums over k), so each core's fT_all is
    rotated so its own 64 columns sit at keys 0..63; Sg is then S[:, 0:64]'s
    class-diagonal 16-blocks, read back from the bf16 S tile itself (bias and
    S agree bitwise, so the k==j relu terms cancel exactly).

Sharding: batch-parallel.  Core m owns 4 classes = outputs[32m:32m+32] and
targets[32m:32m+32] (6.4MB of the 51.4MB input).

Phase 1 (memory-bound): each core sum-pools its 6.4MB shard over the 7x7
window (f32), L2-normalizes each row on-device (Square-accumulate ->
partition-group adds -> Rsqrt -> per-partition scale, one bf16 round), and
transposes to feature-major reference-interleaved order with PE matmuls
against one-hot permutation matrices.  Out: fT [128 d_local, (4 g, 64 col)]
bf16.  Input DMA tops out at ~205 GB/s/core (the per-SENG SDMA budget split
across the core pair) no matter the descriptor path, so phase 1 is pinned at
~31us of DMA + overheads; the normalize/transpose tail adds ~2us, with the
outputs branch handled early in the DMA shadow.

Phase 2: every core gets the rotated fT_all [4, 128, 512] (512KB), computes
its S slice with the own-columns duplicated on both psum partition halves
(one matmul writes S onto 128 partitions -> the 16 j-slots split 8/8 across
halves), extracts the own-class bias columns from the bf16 S, and runs the
raw rank sums spread over DVE/GpSimd/ACT; the host applies +1/division/total.

A single-launch variant with an in-kernel AllGather measured 133.7us:
collectives on this runtime have a ~100us+ latency floor (the nrt BARRIER cc
alone is ~35us and the sem handoff adds ~50us).  A remote_dma_broadcast XOR
all-gather works (logical l>=4 maps to physical tpb l^2, so a logical delta
g>=4 needs physical delta g^2 on broadcast slot g), but without a collective
the 8 cores' launches skew by ~4.4ms under the profiler, which a cross-core
wait absorbs into core 0's measured time.  So the f exchange goes through
the host instead (two NEFF launches at ~13us fixed overhead each).

Measured (neuron-profile exec_time_ns, core 0): see git history of this
docstring; the shared machine drifts between a fast and a ~15% slower mode.
"""

import numpy as np
import ml_dtypes

import concourse.bass as bass
import concourse.bacc as bacc
import concourse.mybir as mybir
import concourse.tile as tile
from concourse.bass_utils import run_bass_kernel_spmd

F32 = mybir.dt.float32
BF16 = mybir.dt.bfloat16
NCORES = 8
BATCH, FEAT, HW = 256, 512, 49
BPC = BATCH // NCORES          # 32 batch rows per branch per core
GROUP, B2 = 16, 512

# pooling chunk widths (c_local units); outputs loads first, targets' last
# chunk is small so the pooling tail after the final DMA is short
O_CHUNKS = [64, 64]
T_CHUNKS = [56, 56, 8, 8]


def build_phase1(dbg=None):
    """Sum-pool + normalize + PE-transpose the shard.
    out: fT bf16 [128 d_local, (4 g, 64 col)] in reference column order."""
    nc = bacc.Bacc("TRN2", target_bir_lowering=False, debug=False,
                   num_devices=NCORES)
    f32 = F32
    AX = mybir.AxisListType
    AF = mybir.ActivationFunctionType
    ALU = mybir.AluOpType
    x_out = nc.dram_tensor("x_out", [BPC, FEAT, HW], f32, kind="ExternalInput")
    x_tgt = nc.dram_tensor("x_tgt", [BPC, FEAT, HW], f32, kind="ExternalInput")
    perm_d = nc.dram_tensor("perm", [2, 64, 128], BF16, kind="ExternalInput")
    fT_d = nc.dram_tensor("fT", [128, 256], BF16, kind="ExternalOutput")
    sg_d = nc.dram_tensor("sg", [64, 64], BF16, kind="ExternalOutput")

    with tile.TileContext(nc) as tc, (
            tc.tile_pool(name="sb", bufs=1)) as sb, (
            tc.tile_pool(name="ps", bufs=1, space="PSUM")) as ps:
        xo = sb.tile([128, 6272], f32, tag="xo")
        xt = sb.tile([128, 6272], f32, tag="xt")
        pooled_o = sb.tile([128, 128], f32, tag="pooled_o")
        pooled_t = sb.tile([128, 128], f32, tag="pooled_t")
        # block-diagonal perm (g-pair P handles groups 2P, 2P+1 in one
        # 64-contraction matmul; PE base partitions must be 0/32/64),
        # replicated on both partition halves
        perm = sb.tile([128, 256], BF16, tag="perm")  # [:, 128b] = branch b
        for P in range(2):
            nc.sync.dma_start(
                perm[64 * P:64 * (P + 1), :].rearrange("p (b n) -> p b n", b=2),
                perm_d.ap().rearrange("b p n -> p b n"))
        warm = sb.tile([32, 2], f32, tag="warm")

        # partition p = 32g + b; row (g,b) holds x[b, 128g:128g+128, :] flat.
        # g=0,1 (partitions 0-63) ride the sync HWDGE ring, g=2,3 ride scalar,
        # which spreads the load over all 16 SDMA engines.  The reduce
        # accumulates in f32.
        def load_chunks(t_, x_, p_, widths):
            c0 = 0
            for w in widths:
                for g in range(4):
                    eng = nc.sync if g < 2 else nc.scalar
                    eng.dma_start(
                        t_[32 * g:32 * (g + 1), 49 * c0:49 * (c0 + w)],
                        x_.ap()[:, g * 128 + c0:g * 128 + c0 + w, :])
                nc.vector.reduce_sum(
                    p_[:, c0:c0 + w],
                    t_[:, 49 * c0:49 * (c0 + w)].rearrange(
                        "p (c h) -> p c h", h=HW),
                    axis=AX.X)
                c0 += w

        def normalize(p_, pnb, sq, ss, inv, br):
            # row norms: per-partition sum of squares, then combine the 4
            # partition groups (g) of each instance, rsqrt, scale+bf16 round
            nc.scalar.activation(sq[:, :], p_[:, :], AF.Square,
                                 accum_out=ss[:, 0:1])
            # two-SB-input ops need equal base partitions, so gather the 4
            # partition-group partials into one free axis and reduce there
            nc.vector.tensor_copy(ss[0:32, 1:2], ss[32:64, 0:1])
            nc.gpsimd.tensor_copy(ss[0:32, 2:3], ss[64:96, 0:1])
            nc.vector.tensor_copy(ss[0:32, 3:4], ss[96:128, 0:1])
            nc.vector.reduce_sum(inv[0:32, 1:2], ss[0:32, 0:4],
                                 axis=AX.X)
            nc.scalar.activation(inv[0:32, 1:2], inv[0:32, 1:2], AF.Sqrt)
            nc.vector.reciprocal(inv[0:32, 0:1], inv[0:32, 1:2])
            nc.vector.tensor_copy(inv[32:64, 0:1], inv[0:32, 0:1])
            nc.vector.tensor_copy(inv[64:128, 0:1], inv[0:64, 0:1])
            with nc.allow_low_precision("single bf16 round of f"):
                nc.vector.tensor_scalar_mul(pnb[:, :], p_[:, :], inv[:, 0:1])
            # transpose to [d_local, col] and interleave via one-hot perm
            for P in range(2):
                nc.tensor.matmul(ps_f[P][:, :], pnb[64 * P:64 * (P + 1), :],
                                 perm[64 * P:64 * (P + 1),
                                      128 * br:128 * (br + 1)],
                                 start=(br == 0), stop=(br == 1))

        ps_f = [ps.tile([128, 128], f32, tag=f"ps_f{P}", name=f"psf{P}")
                for P in range(2)]
        sq_o = sb.tile([128, 128], f32, tag="sq_o")
        sq_t = sb.tile([128, 128], f32, tag="sq_t")
        ss_o = sb.tile([128, 4], f32, tag="ss_o")
        ss_t = sb.tile([128, 4], f32, tag="ss_t")
        inv_o = sb.tile([128, 2], f32, tag="inv_o")
        inv_t = sb.tile([128, 2], f32, tag="inv_t")
        pnb_o = sb.tile([128, 128], BF16, tag="pnb_o")
        pnb_t = sb.tile([128, 128], BF16, tag="pnb_t")
        fT = sb.tile([128, 256], BF16, tag="fT")

        load_chunks(xo, x_out, pooled_o, O_CHUNKS)
        # ACT table preloads (Square/Sqrt) ride in the transfer shadow: the
        # HWDGE queues are saturated with chunk-0 data anyway, and sourcing
        # from a memset tile keeps them off every DMA dependence chain
        nc.vector.memset(warm[:, 0:1], 1.0)
        nc.scalar.activation(warm[:, 1:2], warm[:, 0:1], AF.Square)
        nc.scalar.activation(warm[:, 1:2], warm[:, 0:1], AF.Sqrt)
        # outputs-branch normalize+transpose runs in the targets-DMA shadow
        normalize(pooled_o, pnb_o, sq_o, ss_o, inv_o, 0)
        load_chunks(xt, x_tgt, pooled_t, T_CHUNKS)
        normalize(pooled_t, pnb_t, sq_t, ss_t, inv_t, 1)

        with nc.allow_low_precision("psum f32 -> bf16 fT"):
            nc.vector.tensor_copy(fT[:, 0:128], ps_f[0][:, :])
            nc.scalar.copy(fT[:, 128:256], ps_f[1][:, :])
        nc.sync.dma_start(fT_d.ap()[0:64, :], fT[0:64, :])
        nc.scalar.dma_start(fT_d.ap()[64:128, :], fT[64:128, :])
        # own-block Gram for phase 2's rank biases: same matmul structure
        # (4 g-blocks accumulated in order) as phase 2's S, so the f32 psum
        # and its bf16 round match phase 2's S bitwise
        ps_sg = ps.tile([64, 64], f32, tag="ps_sg", name="ps_sg")
        for g in range(4):
            nc.tensor.matmul(ps_sg[:, :], fT[:, 64 * g:64 * (g + 1)],
                             fT[:, 64 * g:64 * (g + 1)],
                             start=(g == 0), stop=(g == 3))
        sg = sb.tile([64, 64], BF16, tag="sg")
        with nc.allow_low_precision("psum f32 -> bf16 sg"):
            nc.vector.tensor_copy(sg[:, :], ps_sg[:, :])
        nc.sync.dma_start(sg_d.ap()[:, :], sg[:, :])
    nc.compile()
    return nc


def build_phase2(dbg=None):
    """S slice + raw rank sums from the rotated bf16 f^T (own cols at 0..63).
    in: fT_all [4, 128, 512] bf16; out: racc [128, 16] f32
    (cols 0-7 r_all slots, 8-15 r_pos slots)."""
    nc = bacc.Bacc("TRN2", target_bir_lowering=False, debug=False,
                   num_devices=NCORES)
    f32 = F32
    AF = mybir.ActivationFunctionType
    ALU = mybir.AluOpType
    fT_all = nc.dram_tensor("fT_all", [4, 128, 512], BF16,
                            kind="ExternalInput")
    sg_d = nc.dram_tensor("sg", [64, 64], BF16, kind="ExternalInput")
    cmask_d = nc.dram_tensor("cmask", [128, 64], F32, kind="ExternalInput")
    out_d = nc.dram_tensor("out", [128, 16], f32, kind="ExternalOutput")

    with tile.TileContext(nc) as tc, (
            tc.tile_pool(name="sb", bufs=1)) as sb, (
            tc.tile_pool(name="ps", bufs=1, space="PSUM")) as ps:
        ccin = sb.tile([128, 512], BF16, tag="ccin")   # free = (g, col-dup)
        rhs = sb.tile([128, 2048], BF16, tag="rhs")    # free = (g, key)
        warm = sb.tile([128, 1], BF16, tag="warm")
        cmask = sb.tile([128, 64], F32, tag="cmask")
        sgin = sb.tile([64, 64], BF16, tag="sgin")
        nc.sync.dma_start(cmask[:, :], cmask_d.ap())
        nc.scalar.dma_start(sgin[:, :], sg_d.ap())

        # own-columns (keys 0..63 of each g) duplicated for the lhsT, then
        # the full key set; even/odd partition halves ride the two HWDGE
        # rings, g-granular so the first matmuls can start early
        nc.sync.dma_start(
            ccin[:, :].rearrange("p (g two d) -> p g two d", g=4, two=2)[:, :, 0, :],
            fT_all.ap()[:, :, 0:64].rearrange("g p d -> p g d"))
        nc.scalar.dma_start(
            ccin[:, :].rearrange("p (g two d) -> p g two d", g=4, two=2)[:, :, 1, :],
            fT_all.ap()[:, :, 0:64].rearrange("g p d -> p g d"))
        for g in range(4):
            nc.sync.dma_start(rhs[0:64, 512 * g:512 * (g + 1)],
                              fT_all.ap()[g, 0:64, :])
            nc.scalar.dma_start(rhs[64:128, 512 * g:512 * (g + 1)],
                                fT_all.ap()[g, 64:128, :])


        # S slice duplicated by the PE itself: lhsT columns are the own block
        # twice, so psum partitions 0-63 and 64-127 both hold S [64, 512].
        # Two key-half psum tiles = two banks, so the two staging copies
        # below can run on ACT and DVE concurrently (Tile serializes any two
        # engines on one PSUM bank, even read-read).
        # bias prep in the DMA shadow: replicate the own-block Gram rows to
        # both query partition halves, select each query's class block with
        # the 0/1 mask, negate into the 8 per-half j-slot biases, and run
        # the tiny r_pos sums -- all before S exists
        warmm = sb.tile([128, 1], F32, tag="warmm")
        nc.vector.memset(warmm[:, 0:1], 1.0)
        nc.scalar.activation(warm[:, 0:1], warmm[:, 0:1], AF.Relu)
        sgf = sb.tile([128, 64], BF16, tag="sgf")
        nc.vector.tensor_copy(sgf[0:64, :], sgin[:, :])
        nc.vector.tensor_copy(sgf[64:128, :], sgin[:, :])
        SgD = sb.tile([128, 16], f32, tag="SgD")
        sgt = sb.tile([128, 64], f32, tag="sgt")
        AX = mybir.AxisListType
        nc.vector.tensor_tensor(sgt[:, :], sgf[:, :], cmask[:, :],
                                op=ALU.mult)
        nc.vector.reduce_sum(SgD[:, :],
                             sgt[:, :].rearrange("p (c t) -> p t c", c=4),
                             axis=AX.X)
        B8 = sb.tile([128, 8], f32, tag="B8")
        nc.vector.tensor_scalar_mul(B8[0:64, :], SgD[0:64, 0:8], -1.0)
        nc.vector.tensor_scalar_mul(B8[64:128, :], SgD[64:128, 8:16], -1.0)
        scrap_p = sb.tile([128, 16], BF16, tag="scrap_p")
        racc = sb.tile([128, 16], f32, tag="racc")
        zeros = sb.tile([128, 512], BF16, tag="zeros")
        nc.vector.memset(zeros[:, :], 0.0)
        for i in range(8):
            nc.vector.scalar_tensor_tensor(
                out=scrap_p[:, :], in0=SgD[:, :], scalar=B8[:, i:i + 1],
                in1=zeros[:, 0:16], op0=ALU.add, op1=ALU.max,
                accum_out=racc[:, 8 + i:9 + i])

        ps_S = [ps.tile([128, 256], f32, tag=f"ps_S{h}", name=f"psS{h}")
                for h in range(2)]
        for g in range(4):
            for h in range(2):
                nc.tensor.matmul(
                    ps_S[h][:, :], ccin[:, 128 * g:128 * (g + 1)],
                    rhs[:, 512 * g + 256 * h:512 * g + 256 * (h + 1)],
                    start=(g == 0), stop=(g == 3))
        Sb = sb.tile([128, 512], BF16, tag="Sb")
        nc.scalar.copy(Sb[:, 0:256], ps_S[0][:, :])
        nc.vector.tensor_copy(Sb[:, 256:512], ps_S[1][:, :])

        scrap_v = sb.tile([128, 512], BF16, tag="scrap_v")
        scrap_a = sb.tile([128, 512], BF16, tag="scrap_a")

        # r_all: 8 j-slot sums over the 512 keys, split DVE/ACT (the biases
        # came bitwise-equal from phase 1, so k==j cancels exactly)
        for i in range(8):
            if i < 4:
                nc.vector.scalar_tensor_tensor(
                    out=scrap_v[:, :], in0=Sb[:, :], scalar=B8[:, i:i + 1],
                    in1=zeros[:, :], op0=ALU.add, op1=ALU.max,
                    accum_out=racc[:, i:i + 1])
            else:
                nc.scalar.activation(
                    scrap_a[:, :], Sb[:, :], AF.Relu, bias=B8[:, i:i + 1],
                    accum_out=racc[:, i:i + 1])
        nc.sync.dma_start(out_d.ap()[0:64, :], racc[0:64, :])
        nc.scalar.dma_start(out_d.ap()[64:128, :], racc[64:128, :])
    nc.compile()
    return nc


_NC1 = None
_NC2 = None


def _get_ncs():
    global _NC1, _NC2
    if _NC1 is None:
        _NC1 = build_phase1()
        _NC2 = build_phase2()
    return _NC1, _NC2


# one-hot column permutation: branch-ordered b -> reference interleaved
# col = 16*(b//8) + 8*branch + b%8
def _perm_mats():
    p = np.zeros((2, 64, 128), ml_dtypes.bfloat16)
    for br in range(2):
        for gg in range(2):
            for b in range(32):
                p[br, 32 * gg + b,
                  64 * gg + 16 * (b // 8) + 8 * br + (b % 8)] = 1
    return p


_PERM = _perm_mats()

# per-partition class selector, broadcast over the 16 in-class columns:
# partition p (query slot) belongs to class (p % 64) // 16
_CMASK = np.zeros((128, 4, 16), np.float32)
for _p in range(128):
    _CMASK[_p, (_p % 64) // 16, :] = 1.0
_CMASK = np.ascontiguousarray(_CMASK.reshape(128, 64))


def make_in_maps1(outputs, targets):
    outputs = np.ascontiguousarray(
        np.asarray(outputs, dtype=np.float32)).reshape(BATCH, FEAT, HW)
    targets = np.ascontiguousarray(
        np.asarray(targets, dtype=np.float32)).reshape(BATCH, FEAT, HW)
    return [
        {
            "x_out": np.ascontiguousarray(outputs[m * BPC:(m + 1) * BPC]),
            "x_tgt": np.ascontiguousarray(targets[m * BPC:(m + 1) * BPC]),
            "perm": _PERM,
        }
        for m in range(NCORES)
    ]


def make_in_maps2(results1):
    """fT [128, (g, col)] bf16 per core -> per-core rotated fT_all
    [4, 128, 512] with the own 64 columns first."""
    blocks = [results1[m]["fT"].reshape(128, 4, 64).transpose(1, 0, 2)
              for m in range(NCORES)]              # [4 g, 128 d, 64 col]
    maps = []
    for m in range(NCORES):
        rot = np.concatenate([blocks[(m + j) % NCORES] for j in range(NCORES)],
                             axis=2)               # [4, 128, 512]
        maps.append({"fT_all": np.ascontiguousarray(rot),
                     "sg": results1[m]["sg"], "cmask": _CMASK})
    return maps


def finish(results2):
    total = 0.0
    for m in range(NCORES):
        racc = results2[m]["out"].astype(np.float64)      # [128, 16]
        total += ((1.0 + racc[:, 8:16]) / (1.0 + racc[:, 0:8])).sum()
    return np.array(1.0 - total / (GROUP * B2), dtype=np.float32)


def kernel(outputs, targets):
    nc1, nc2 = _get_ncs()
    res1 = run_bass_kernel_spmd(nc1, make_in_maps1(outputs, targets),
                                core_ids=list(range(NCORES)))
    res2 = run_bass_kernel_spmd(nc2, make_in_maps2(res1.results),
                                core_ids=list(range(NCORES)))
    return finish(res2.results)


if __name__ == "__main__":
    import reference as ref
    inputs = ref.setup_inputs()
    actual = kernel(**{k: np.asarray(v) for k, v in inputs.items()})
    print("kernel result:", actual)
